# revision 23
# baseline (speedup 1.0000x reference)
"""Trainium2 Bass kernel for nn_AttentionBlock (GroupNorm + qkv conv + head-dim attention + proj + residual).

Sharding: data-parallel over batch B=16 -> 2 batch elements per core on 8 cores.

Math restructure vs the direct formulation:
  scores = Q K^T (contraction over N=4096 pixels) via the Gram matrix
  G_aug = [X;1][X;1]^T computed once in f32r (full PE rate at moving>=256):
  sc_h = M_q G_aug M_k^T with M = [W D_a | W b2 + b_qkv] (GroupNorm folded as
  xn = a*x + b2). Q and K are never materialized. Channel stats (s = X@1,
  sum x^2 = diag(G)) fall out of the Gram pass; group aggregation/broadcast
  uses tiny indicator matmuls.
  The whole attention tail collapses to ONE output GEMM:
    out = F X2B + ob 1^T + X2B,  F^T = D_a Wv^T (WpA)^T,  (WpA)^T = E_norm WpT
  per head, ob = (WpA) vb. V is never materialized. X2B = x + b_proj so the
  proj bias and residual ride the same tensor.
"""
import sys
sys.path.insert(0, "/opt/trn_rl_repo")
sys.path.insert(0, "/opt/trn_rl_repo/concourse")
import numpy as np

B, C, H, W = 16, 512, 64, 64
N = H * W            # 4096 pixels
NH = 8               # heads
D = C // NH          # 64 head dim
EPS = 1e-5
NCORES = 8
BPC = B // NCORES    # 2 batches per core

NT = C // 128        # 4 channel chunks
NJ = N // 512        # 8 pixel blocks of 512
NCH = N // 128       # 32 pixel chunks of 128 (Gram stream)
CA = 514             # xT padded cols: 512 channels + ones col + pad

_cache = {}


def _build():
    import concourse.bass as bass
    import concourse.bacc as bacc
    import concourse.tile as tile
    from concourse import mybir
    from concourse.masks import make_identity

    f32 = mybir.dt.float32
    f32r = mybir.dt.float32r
    bf16 = mybir.dt.bfloat16
    fp16 = mybir.dt.float16
    AF = mybir.ActivationFunctionType
    ALU = mybir.AluOpType
    AX = mybir.AxisListType

    nc = bacc.Bacc()

    fp8 = mybir.dt.float8e4
    x8 = nc.dram_tensor("x8", [BPC, C, N], fp8, kind="ExternalInput")      # fp8(x + b_proj)
    xlo8 = nc.dram_tensor("xlo8", [BPC, C, N], fp8, kind="ExternalInput")  # fp8 of the remainder
    xtf = nc.dram_tensor("xtf", [BPC, N, CA], fp16, kind="ExternalInput")   # x^T | ones | 0
    wqkf = nc.dram_tensor("wqkf", [C, 3 * C], f32, kind="ExternalInput")   # w_qkv.T f32
    wv_dd = nc.dram_tensor("wv_dd", [C, C], bf16, kind="ExternalInput")    # w_qkv v rows, d-major
    wpb_d = nc.dram_tensor("wpb_d", [C, C], bf16, kind="ExternalInput")    # w_proj.T bf16
    gamma_pc = nc.dram_tensor("gamma_pc", [128, NT], f32, kind="ExternalInput")
    beta_pc = nc.dram_tensor("beta_pc", [128, NT], f32, kind="ExternalInput")
    bp_pc = nc.dram_tensor("bp_pc", [128, NT], f32, kind="ExternalInput")
    bqk_row = nc.dram_tensor("bqk_row", [1, 2 * C], f32, kind="ExternalInput")
    bv_row = nc.dram_tensor("bv_row", [1, C], f32, kind="ExternalInput")
    ind_g = nc.dram_tensor("ind_g", [128, 8], f32, kind="ExternalInput")   # p//16 == g
    ind_e = nc.dram_tensor("ind_e", [8, 128], f32, kind="ExternalInput")   # transpose
    out2 = nc.dram_tensor("out2", [BPC, C, N], bf16, kind="ExternalOutput")

    with tile.TileContext(nc) as tc:
        with tc.tile_pool(name="consts", bufs=1) as consts, \
             tc.tile_pool(name="xtp", bufs=6) as xtp, \
             tc.tile_pool(name="xbfp", bufs=1) as xbfp, \
             tc.tile_pool(name="wbp", bufs=2) as wbp, \
             tc.tile_pool(name="gxp", bufs=1) as gxp, \
             tc.tile_pool(name="work", bufs=2) as work, \
             tc.tile_pool(name="stagep", bufs=4) as stagep, \
             tc.tile_pool(name="ps", bufs=1, space="PSUM") as ps:

            # ---------------- constants (once per core) ----------------
            identf = consts.tile([128, 128], f32, tag="identf")
            make_identity(nc, identf)

            def load_consts():
                nc.sync.dma_start(out=gam, in_=gamma_pc[:, :])
                nc.sync.dma_start(out=bet, in_=beta_pc[:, :])
                nc.sync.dma_start(out=bpc, in_=bp_pc[:, :])
                nc.sync.dma_start(out=bqkr, in_=bqk_row[:, :])
                nc.sync.dma_start(out=bvr, in_=bv_row[:, :])
                nc.scalar.dma_start(out=indg, in_=ind_g[:, :])
                nc.scalar.dma_start(out=inde, in_=ind_e[:, :])
                nc.vector.memset(eps8, EPS)

            gam = consts.tile([128, NT], f32, tag="gam")
            bet = consts.tile([128, NT], f32, tag="bet")
            bpc = consts.tile([128, NT], f32, tag="bpc")
            bqkr = consts.tile([1, 2 * C], f32, tag="bqkr")
            bvr = consts.tile([1, C], f32, tag="bvr")
            indg = consts.tile([128, 8], f32, tag="indg")
            inde = consts.tile([8, 128], f32, tag="inde")
            eps8 = consts.tile([8, 1], f32, tag="eps8")
            def load_weights():
                for t in range(NT):
                    w_t = consts.tile([128, 3 * C], f32, tag=f"wqk{t}", name=f"wqk{t}")
                    for j in range(3):
                        (nc.sync if (t + j) % 2 == 0 else nc.scalar).dma_start(
                            out=w_t[:, 512 * j:512 * (j + 1)],
                            in_=wqkf[128 * t:128 * (t + 1), 512 * j:512 * (j + 1)])
                    wqk.append(w_t)
                    v_t = consts.tile([128, C], bf16, tag=f"wvd{t}", name=f"wvd{t}")
                    nc.scalar.dma_start(out=v_t, in_=wv_dd[128 * t:128 * (t + 1), :])
                    wvd.append(v_t)
                    p_t = consts.tile([128, C], bf16, tag=f"wpb{t}", name=f"wpb{t}")
                    nc.scalar.dma_start(out=p_t, in_=wpb_d[128 * t:128 * (t + 1), :])
                    wpb.append(p_t)
            wqk = []   # [128, 1536] f32 per c-chunk (resident)
            wvd = []   # [128, 512] bf16 per d-chunk (v weights, d on partitions)
            wpb = []   # [128, 512] bf16 per c-chunk

            # PSUM banks (8): gram0 | g1s | g23 | mm512 x3 | scq | tiny
            def mm512(name):
                return ps.tile([128, 512], f32, tag="mm512", name=name, bufs=3)

            def tinyps(name, rows=128):
                return ps.tile([rows, 128], f32, tag="tiny", name=name, bufs=1)

            # ---------------- input streams ----------------
            batch_state = [{} for _ in range(BPC)]

            def load_xt(b):
                st = batch_state[b]
                st["xt_tiles"] = []
                engs = [nc.sync, nc.scalar, nc.gpsimd]
                for ch in range(NCH):
                    xc = xtp.tile([128, CA], fp16, tag="xt", name=f"xc{b}_{ch}")
                    engs[ch % 3].dma_start(
                        out=xc, in_=xtf[b, 128 * ch:128 * (ch + 1), :])
                    st["xt_tiles"].append(xc)

            def load_xbf(b):
                st = batch_state[b]
                st["x8dr"] = []
                st["xlodr"] = []
                for P in range(2):
                    for nm, src_t, lst in (("x8", x8, "x8dr"), ("xlo", xlo8, "xlodr")):
                        xd = xbfp.tile([128, 2, N], fp8, tag=f"{nm}dr{P}",
                                       name=f"{nm}dr{P}_{b}")
                        for i in range(2):
                            t = 2 * P + i
                            (nc.sync if (P + i) % 2 == 0 else nc.scalar).dma_start(
                                out=xd[:, i, :],
                                in_=src_t[b, 128 * t:128 * (t + 1), :])
                        st[lst].append(xd)

            # ================= per-batch phases =================
            # Gram PSUM layout (f32r, all moving widths >= 256 for full rate):
            #   bank gram0: gps0 [128,512]  <- moving [0:512]   (block row 0)
            #   bank g1s:   gps1 [0:385]    <- moving [128:513] (block row 1, s1 at col 384)
            #               sps0 [385:386], sps2 [386:387]      (s cols, 4x-rate tiny)
            #   bank g23:   gps2 [0:256]    <- moving [256:512] (block row 2)
            #               gps3 [256:512]  <- moving [257:513] (block row 3, s3 at col 255)
            def gram(b):
                st = batch_state[b]
                gps0 = ps.tile([128, 512], f32, tag="gram0", name=f"gps0_{b}")
                g1s = ps.tile([128, 386], f32, tag="g1s", name=f"g1s_{b}")
                g23 = ps.tile([128, 386], f32, tag="g23", name=f"g23_{b}")
                gps1 = g1s[:, 0:385]
                sps0 = g1s[:, 385:386]
                gps2 = g23[:, 0:257]
                gps3 = g23[:, 257:386]
                mv = [(0, 0, 512), (1, 128, 513), (2, 256, 513), (3, 384, 513)]
                gview = [gps0, gps1, gps2, gps3]
                for ch in range(NCH):
                    xc = st["xt_tiles"][ch]
                    for i, lo, hi in mv:
                        nc.tensor.matmul(gview[i], xc[:, 128 * i:128 * (i + 1)],
                                         xc[:, lo:hi],
                                         start=(ch == 0 and i != 3),
                                         stop=(ch == NCH - 1 and i != 2),
                                         skip_group_check=True)
                    nc.tensor.matmul(sps0, xc[:, 0:128], xc[:, 512:513],
                                     start=False, stop=(ch == NCH - 1),
                                     skip_group_check=True)
                st["gview"] = gview
                st["scols"] = [sps0, gps1[:, 384:385], gps2[:, 256:257],
                               gps3[:, 128:129]]
                # diagonals all start at [p, p] of each block view
                st["dviews"] = [gps0, gps1, gps2, gps3]

            def stats(b):
                st = batch_state[b]
                import concourse.bass as _bass

                def diag_ap(sl):
                    pitch = sl.ap[0][0]
                    return _bass.AP(tensor=sl.tensor, offset=sl.offset,
                                    ap=[[pitch + 1, 128], [1, 1]])

                stat = work.tile([128, 8], f32, tag="stat")  # cols: s x4, sumsq x4
                for i in range(NT):
                    nc.vector.tensor_copy(stat[:, i:i + 1], st["scols"][i])
                for i in range(NT):
                    nc.vector.tensor_copy(stat[:, 4 + i:5 + i], diag_ap(st["dviews"][i]))

                gstat_ps = tinyps(f"gstat_{b}", rows=8)
                nc.tensor.matmul(gstat_ps[:, 0:8], indg.bitcast(f32r),
                                 stat.bitcast(f32r), start=True, stop=True)
                gstat = work.tile([8, 8], f32, tag="gstat")
                nc.vector.tensor_copy(gstat, gstat_ps[:, 0:8])
                mr = work.tile([8, 8], f32, tag="mr")  # cols: mean x4, rstd x4
                nc.vector.tensor_scalar(out=mr, in0=gstat,
                                        scalar1=1.0 / 65536.0, scalar2=None, op0=ALU.mult)
                msq = work.tile([8, 4], f32, tag="msq")
                nc.vector.tensor_tensor(msq, mr[:, 0:4], mr[:, 0:4], op=ALU.mult)
                nc.vector.tensor_tensor(mr[:, 4:8], mr[:, 4:8], msq, op=ALU.subtract)
                lnv = work.tile([8, 4], f32, tag="lnv")
                nc.scalar.activation(out=lnv, in_=mr[:, 4:8], func=AF.Ln, bias=eps8)
                nc.scalar.activation(out=mr[:, 4:8], in_=lnv, func=AF.Exp, scale=-0.5)
                pc_ps = tinyps(f"pc_{b}")
                nc.tensor.matmul(pc_ps[:, 0:8], inde.bitcast(f32r), mr.bitcast(f32r),
                                 start=True, stop=True)
                pc = work.tile([128, 8], f32, tag="pc")
                nc.vector.tensor_copy(pc, pc_ps[:, 0:8])
                mean_pc, rstd_pc = pc[:, 0:4], pc[:, 4:8]

                acol = work.tile([128, NT], f32, tag="acol")
                nc.vector.tensor_tensor(acol, rstd_pc, gam, op=ALU.mult)
                tmp = work.tile([128, NT], f32, tag="tmpb")
                nc.vector.tensor_tensor(tmp, mean_pc, acol, op=ALU.mult)
                b2 = work.tile([128, NT], f32, tag="b2")
                nc.vector.tensor_tensor(b2, bet, tmp, op=ALU.subtract)
                # the final GEMM consumes x+bp, so vb gets a -a*bp correction
                abp = work.tile([128, NT], f32, tag="abp")
                nc.vector.tensor_tensor(abp, acol, bpc, op=ALU.mult)
                st["acol"] = acol
                scolf = work.tile([128, NT], f32, tag="scolf")
                nc.vector.tensor_copy(scolf, stat[:, 0:4])
                st["scolf"] = scolf

                # scaled q/k weights, f32 (exact score path)
                wsb = []
                for t in range(NT):
                    w_t = wbp.tile([128, 2 * C], f32, tag=f"wsb{t}", name=f"wsb{t}_{b}", bufs=1)
                    nc.vector.tensor_scalar_mul(out=w_t, in0=wqk[t][:, 0:2 * C],
                                                scalar1=acol[:, t:t + 1])
                    wsb.append(w_t)
                st["wsb"] = wsb
                wsbh = []
                for t in range(NT):
                    w_h = wbp.tile([128, C], fp16, tag=f"wsbh{t}", name=f"wsbh{t}_{b}",
                                   bufs=1)
                    nc.vector.tensor_copy(w_h, wsb[t][:, 0:512])
                    wsbh.append(w_h)
                st["wsbh"] = wsbh

                # bias rows: [Wq b2 + bq | Wk b2 + bk], v bias row (minus a*bp term)
                qkb2 = work.tile([1, 2 * C], f32, tag="qkb2")
                vbrow = work.tile([1, C], f32, tag="vbrow")
                for j in range(3):
                    bps = mm512(f"b2r{j}_{b}")
                    for t in range(NT):
                        nc.tensor.matmul(bps[0:1, :], b2.bitcast(f32r)[:, t:t + 1],
                                         wqk[t].bitcast(f32r)[:, 512 * j:512 * (j + 1)],
                                         start=(t == 0), stop=(t == NT - 1))
                    if j < 2:
                        nc.vector.tensor_add(qkb2[:, 512 * j:512 * (j + 1)],
                                             bps[0:1, :], bqkr[:, 512 * j:512 * (j + 1)])
                    else:
                        nc.vector.tensor_add(vbrow, bps[0:1, :], bvr)
                vcps = mm512(f"vcorr_{b}")
                for t in range(NT):
                    nc.tensor.matmul(vcps[0:1, :], abp.bitcast(f32r)[:, t:t + 1],
                                     wqk[t].bitcast(f32r)[:, 2 * C:3 * C],
                                     start=(t == 0), stop=(t == NT - 1))
                nc.vector.tensor_tensor(vbrow, vbrow, vcps[0:1, :], op=ALU.subtract)
                st["qkb2"] = qkb2
                qkb2h = work.tile([1, C], fp16, tag="qkb2h")
                nc.vector.tensor_copy(qkb2h, qkb2[:, 0:512])
                st["qkb2h"] = qkb2h
                # vb as bf16 per-partition columns (for ob = WpA vb)
                vbcb = work.tile([128, NT], bf16, tag="vbcb")
                for m in range(NT):
                    tp = tinyps(f"vbt{m}_{b}")
                    nc.tensor.transpose(tp[:, 0:1], vbrow[:, 128 * m:128 * (m + 1)],
                                        identf[0:1, 0:1])
                    nc.vector.tensor_copy(vbcb[:, m:m + 1], tp[:, 0:1])
                st["vbcb"] = vbcb
                # s as f32 row (for the rank-1 Gram-augmentation terms)
                srow = work.tile([1, C], f32, tag="srow")
                for i in range(NT):
                    tp = tinyps(f"srt{i}_{b}", rows=1)
                    nc.tensor.transpose(tp, stat[:, i:i + 1], identf)
                    nc.vector.tensor_copy(srow[:, 128 * i:128 * (i + 1)], tp)
                st["srow"] = srow

                # Gx psum -> f32 row-tiles (full square via symmetry)
                gxr = [gxp.tile([128, 512], f32, tag=f"gxr{i}", name=f"gxr{i}_{b}")
                       for i in range(NT)]
                gview = st["gview"]
                off = [0, 128, 256, 384]
                for i in range(NT):
                    for j in range(i, NT):
                        so = 128 * j - off[i]
                        nc.vector.tensor_copy(gxr[i][:, 128 * j:128 * (j + 1)],
                                              gview[i][:, so:so + 128])
                for j in range(NT):
                    for i in range(j + 1, NT):  # lower triangle: transpose block (j,i)
                        tp = tinyps(f"gxt{i}{j}_{b}")
                        nc.tensor.transpose(tp.bitcast(f32r),
                                            gxr[j].bitcast(f32r)[:, 128 * i:128 * (i + 1)],
                                            identf.bitcast(f32r))
                        nc.vector.tensor_copy(gxr[i][:, 128 * j:128 * (j + 1)], tp)
                st["gxr"] = gxr

            def t2_sc(b):
                st = batch_state[b]
                gxr, wsb, qkb2, srow, scolf = (st["gxr"], st["wsb"], st["qkb2"],
                                               st["srow"], st["scolf"])
                t2b = []
                for a in range(NT):
                    t2_ps = mm512(f"t2_{a}_{b}")
                    for cb in range(NT):
                        nc.tensor.matmul(t2_ps, gxr[cb].bitcast(f32r)[:, 128 * a:128 * (a + 1)],
                                         wsb[cb].bitcast(f32r)[:, 512:1024],
                                         start=(cb == 0), stop=False)
                    nc.tensor.matmul(t2_ps, srow.bitcast(f32r)[:, 128 * a:128 * (a + 1)],
                                     qkb2.bitcast(f32r)[:, 512:1024], start=False, stop=True)
                    t2_t = work.tile([128, 512], fp16, tag=f"t2b{a}", bufs=1)
                    nc.vector.tensor_copy(t2_t, t2_ps)
                    t2b.append(t2_t)
                t2r_ps = mm512(f"t2r_{b}")
                for cb in range(NT):
                    nc.tensor.matmul(t2r_ps[0:1, :], scolf.bitcast(f32r)[:, cb:cb + 1],
                                     wsb[cb].bitcast(f32r)[:, 512:1024],
                                     start=(cb == 0), stop=(cb == NT - 1))
                t2rf = work.tile([1, 512], f32, tag="t2rf")
                nc.vector.tensor_scalar(out=t2rf, in0=qkb2[:, 512:1024],
                                        scalar1=float(N), scalar2=None, op0=ALU.mult)
                nc.vector.tensor_tensor(t2rf, t2rf, t2r_ps[0:1, :], op=ALU.add)
                t2rh = work.tile([1, 512], fp16, tag="t2rh")
                nc.vector.tensor_copy(t2rh, t2rf)
                wsbh = st["wsbh"]
                qkb2h = st["qkb2h"]

                # one accumulation group for the whole packed scp bank
                scp = ps.tile([128, 512], f32, tag="scq", name=f"scp_{b}")
                for h in range(NH):
                    p, r = h // 2, (h % 2) * 64
                    out_ap = scp[r:r + 64, 64 * p:64 * (p + 1)]
                    for a in range(NT):
                        nc.tensor.matmul(out_ap, wsbh[a][:, 64 * h:64 * h + 64],
                                         t2b[a][:, 64 * h:64 * h + 64],
                                         start=(h < 2 and a == 0), stop=False,
                                         skip_group_check=True)
                    nc.tensor.matmul(out_ap, qkb2h[:, 64 * h:64 * h + 64],
                                     t2rh[:, 64 * h:64 * h + 64],
                                     start=False, stop=(h >= NH - 2),
                                     skip_group_check=True)
                st["scp"] = scp

            def softmax(b):
                st = batch_state[b]
                scp = st["scp"]
                ebs = []
                for p in range(NT):
                    sl = scp[:, 64 * p:64 * (p + 1)]
                    mx = work.tile([128, 1], f32, tag="mx")
                    nc.vector.reduce_max(out=mx, in_=sl, axis=AX.X)
                    negmx = work.tile([128, 1], f32, tag="negmx")
                    nc.vector.tensor_scalar(out=negmx, in0=mx, scalar1=-0.125,
                                            scalar2=None, op0=ALU.mult)
                    e = work.tile([128, 64], f32, tag="exp")
                    nc.scalar.activation(out=e, in_=sl, func=AF.Exp,
                                         scale=0.125, bias=negmx)
                    den = work.tile([128, 1], f32, tag="den")
                    nc.vector.reduce_sum(out=den, in_=e, axis=AX.X)
                    rden = work.tile([128, 1], f32, tag="rden")
                    nc.vector.reciprocal(rden, den)
                    eb = work.tile([128, 64], bf16, tag=f"eb{p}")
                    nc.scalar.activation(out=eb, in_=e, func=AF.Copy,
                                         scale=rden[:, 0:1])
                    ebs.append(eb)
                st["ebs"] = ebs

            def fgen(b):
                st = batch_state[b]
                ebs, acol, vbcb = st["ebs"], st["acol"], st["vbcb"]
                # (WpA)^T per d-chunk -> sbuf bf16
                wpat_sb = []
                for dc in range(NT):
                    w_ps = mm512(f"wpat{dc}_{b}")
                    for hh in range(2):
                        r = hh * 64
                        nc.tensor.matmul(w_ps[r:r + 64, :], ebs[dc][r:r + 64, :],
                                         wpb[dc][r:r + 64, :], start=True, stop=True,
                                         skip_group_check=True)
                    w_sb = work.tile([128, 512], bf16, tag=f"wpat_sb{dc}")
                    nc.scalar.activation(out=w_sb, in_=w_ps, func=AF.Copy)
                    wpat_sb.append(w_sb)
                # ob row = vb^T WpAT  (accumulate over d-chunks)
                ob_ps = mm512(f"ob_{b}")
                for dc in range(NT):
                    nc.tensor.matmul(ob_ps[0:1, :], vbcb[:, dc:dc + 1], wpat_sb[dc],
                                     start=(dc == 0), stop=(dc == NT - 1))
                obrow = work.tile([1, C], f32, tag="obrow")
                nc.vector.tensor_copy(obrow, ob_ps[0:1, :])
                obc = work.tile([128, NT], f32, tag="obc")
                for m in range(NT):
                    tp = tinyps(f"obt{m}_{b}")
                    nc.tensor.transpose(tp[:, 0:1], obrow[:, 128 * m:128 * (m + 1)],
                                        identf[0:1, 0:1])
                    nc.vector.tensor_copy(obc[:, m:m + 1], tp[:, 0:1])
                st["obc"] = obc
                # F'^T = D_a (Wv^T WpAT) + I (identity folds the residual into
                # the GEMM), split F' = F8 + Flo, both fp8 in DoubleRow layout
                f8dr = [wbp.tile([128, 2, 512], fp8, tag=f"f8dr{P}", name=f"f8dr{P}_{b}")
                        for P in range(2)]
                flodr = [wbp.tile([128, 2, 512], fp8, tag=f"flodr{P}", name=f"flodr{P}_{b}")
                         for P in range(2)]
                import concourse.bass as _bass
                for cb in range(NT):
                    h_ps = mm512(f"h_{cb}_{b}")
                    for dc in range(NT):
                        nc.tensor.matmul(h_ps, wvd[dc][:, 128 * cb:128 * (cb + 1)],
                                         wpat_sb[dc], start=(dc == 0), stop=(dc == NT - 1))
                    ftmp = work.tile([128, 512], f32, tag="ftmp")
                    nc.vector.tensor_scalar_mul(out=ftmp, in0=h_ps,
                                                scalar1=acol[:, cb:cb + 1])
                    dg = _bass.AP(tensor=ftmp.tensor, offset=ftmp.offset + 128 * cb,
                                  ap=[[513, 128], [1, 1]])
                    nc.vector.tensor_scalar(out=dg, in0=dg, scalar1=1.0,
                                            scalar2=None, op0=ALU.add)
                    P, i = cb // 2, cb % 2
                    nc.scalar.activation(out=f8dr[P][:, i, :], in_=ftmp, func=AF.Copy)
                    nc.gpsimd.tensor_tensor(flodr[P][:, i, :], ftmp, f8dr[P][:, i, :],
                                            op=ALU.subtract)
                st["f8dr"] = f8dr
                st["flodr"] = flodr

            def fx(b, nj_lo=0, nj_hi=NJ):
                st = batch_state[b]
                f8dr, flodr, x8dr, xlodr, obc = (st["f8dr"], st["flodr"], st["x8dr"],
                                                 st["xlodr"], st["obc"])
                DR = mybir.MatmulPerfMode.DoubleRow
                k = nj_lo * NT
                for nj in range(nj_lo, nj_hi):
                    for m in range(NT):
                        pps = mm512(f"pps{m}_{nj}_{b}")
                        terms = [(f8dr, x8dr), (flodr, x8dr), (f8dr, xlodr)]
                        for ti, (fT, xT) in enumerate(terms):
                            for P in range(2):
                                nc.tensor.matmul(
                                    pps, fT[P][:, :, 128 * m:128 * (m + 1)],
                                    xT[P][:, :, 512 * nj:512 * (nj + 1)],
                                    start=(ti == 0 and P == 0),
                                    stop=(ti == 2 and P == 1), perf_mode=DR)
                        stage = stagep.tile([128, 512], bf16, tag="stage")
                        if k % 2 == 0:
                            nc.vector.tensor_scalar(out=stage, in0=pps,
                                                    scalar1=obc[:, m:m + 1],
                                                    scalar2=None, op0=ALU.add)
                        else:
                            nc.gpsimd.tensor_scalar(out=stage, in0=pps,
                                                    scalar1=obc[:, m:m + 1],
                                                    scalar2=None, op0=ALU.add)
                        k += 1
                        (nc.sync if k % 2 == 0 else nc.scalar).dma_start(
                            out=out2[b, 128 * m:128 * (m + 1), 512 * nj:512 * (nj + 1)],
                            in_=stage)

            # ================= pipeline =================
            load_xt(0)
            load_consts()
            load_weights()
            load_xt(1)
            load_xbf(0)
            gram(0)
            stats(0)
            t2_sc(0)
            softmax(0)
            gram(1)       # PE fills the softmax gap of batch 0
            fgen(0)
            stats(1)      # DVE work overlaps fgen PE work
            t2_sc(1)      # PE covers batch 0's F-quantization chain
            softmax(1)
            fx(0, 0, 2)
            load_xbf(1)
            fgen(1)       # its F-chain overlaps the rest of fx(0)
            fx(0, 2, NJ)
            fx(1)

    nc.compile()
    return nc


def _get_nc():
    if "nc" not in _cache:
        _cache["nc"] = _build()
    return _cache["nc"]


def make_core_inputs(x, gamma, beta, w_qkv, b_qkv, w_proj, b_proj):
    """Host-side prep: returns the list of per-core input dicts."""
    import ml_dtypes
    bf = ml_dtypes.bfloat16

    x = np.asarray(x, dtype=np.float32).reshape(B, C, N)
    gamma = np.asarray(gamma, dtype=np.float32)
    beta = np.asarray(beta, dtype=np.float32)
    w_qkv = np.asarray(w_qkv, dtype=np.float32)
    b_qkv = np.asarray(b_qkv, dtype=np.float32)
    w_proj = np.asarray(w_proj, dtype=np.float32)
    b_proj = np.asarray(b_proj, dtype=np.float32)

    f8 = ml_dtypes.float8_e4m3
    x2b_full = x + b_proj[None, :, None]                      # proj bias rides resid
    x8_full = x2b_full.astype(f8)
    xlo8_full = (x2b_full - x8_full.astype(np.float32)).astype(f8)
    xtf_full = np.zeros((B, N, CA), dtype=np.float16)
    xtf_full[:, :, 0:512] = x.transpose(0, 2, 1).astype(np.float16)
    xtf_full[:, :, 512] = 1.0

    wqkf = np.ascontiguousarray(w_qkv.T)                      # [512, 1536] f32
    wv_d = np.ascontiguousarray(w_qkv[2 * C:].astype(bf))     # [512 d, 512 c] bf16
    wpb = np.ascontiguousarray(w_proj.T.astype(bf))           # [512, 512] bf16
    gamma_pc = np.ascontiguousarray(gamma.reshape(NT, 128).T)
    beta_pc = np.ascontiguousarray(beta.reshape(NT, 128).T)
    bp_pc = np.ascontiguousarray(b_proj.reshape(NT, 128).T)
    bqk_row = np.ascontiguousarray(b_qkv[:2 * C].reshape(1, 2 * C))
    bv_row = np.ascontiguousarray(b_qkv[2 * C:].reshape(1, C))
    p_idx = np.arange(128)
    ind_g = (p_idx[:, None] // 16 == np.arange(8)[None, :]).astype(np.float32)
    ind_e = np.ascontiguousarray(ind_g.T)

    in_maps = []
    for i in range(NCORES):
        in_maps.append({
            "x8": np.ascontiguousarray(x8_full[BPC * i:BPC * (i + 1)]),
            "xlo8": np.ascontiguousarray(xlo8_full[BPC * i:BPC * (i + 1)]),
            "xtf": np.ascontiguousarray(xtf_full[BPC * i:BPC * (i + 1)]),
            "wqkf": wqkf, "wv_dd": wv_d, "wpb_d": wpb,
            "gamma_pc": gamma_pc, "beta_pc": beta_pc, "bp_pc": bp_pc,
            "bqk_row": bqk_row, "bv_row": bv_row,
            "ind_g": ind_g, "ind_e": ind_e,
        })
    return in_maps


def kernel(x, gamma, beta, w_qkv, b_qkv, w_proj, b_proj):
    from concourse.bass_utils import run_bass_kernel_spmd

    nc = _get_nc()
    in_maps = make_core_inputs(x, gamma, beta, w_qkv, b_qkv, w_proj, b_proj)
    res = run_bass_kernel_spmd(nc, in_maps, core_ids=list(range(NCORES)))
    out = np.empty((B, C, N), dtype=np.float32)
    for i in range(NCORES):
        out[BPC * i:BPC * (i + 1)] = np.asarray(res.results[i]["out2"], dtype=np.float32)
    return out.reshape(B, C, H, W)
